# revision 33
# baseline (speedup 1.0000x reference)
"""Causal multi-head attention on 8 Trainium2 NeuronCores.

Problem: x[2,4096,512], W_q/W_k/W_v/W_proj[512,512], b_proj[512]
  q,k,v = x @ W.T split into 8 heads of 64; causal softmax(q k^T / 8) v;
  out = attn @ W_proj.T + b_proj.

Sharding: 16 (batch, head) pairs over 8 cores -> each core gets one batch
and a pair of adjacent heads (128 of the 512 hidden dims).  The output
projection is computed per-core against the matching 128-row slice of
W_proj^T, giving a partial [4096, 512] output per core; the host sums the
4 partials per batch and adds the bias.

On-core layout (all "T" = transposed so the contraction dim sits on SBUF
partitions):
  xT   [512, 4096]  (host-transposed input slice)
  qT/kT_pair [128, 4096]  rows 0-63 head0, 64-127 head1
  scoresT blocks [128 k, 512 q]  = k-block^T q  (row-tiled 2 heads on PE)
  exp via ScalarE (scale=1/8 folded in), causal mask via gpsimd
  attnT accumulated as [65, 512] per head: rows 0-63 v^T exp, row 64 the
  softmax denominators (ones column appended to v).
"""

import numpy as np

B, S, D, H = 2, 4096, 512, 8
DH = 64
QCHUNK = 512
SCALE = 1.0 / np.sqrt(DH)

_CACHE = {}


def _build(s=S, normalize=True, debug_dumps=False, repeats=1):
    from contextlib import ExitStack

    import concourse.mybir as mybir
    import concourse.tile as tile
    from concourse import bacc
    from concourse.masks import make_identity

    f32 = mybir.dt.float32
    f32r = mybir.dt.float32r
    EXP = mybir.ActivationFunctionType.Exp
    GE = mybir.AluOpType.is_ge

    nqc = s // QCHUNK      # q chunks
    nkb_all = s // 128     # k blocks
    ndc = D // 128         # D chunks (contraction for projections)
    kb_per_chunk = QCHUNK // 128

    nc = bacc.Bacc("TRN2")
    xT_d = nc.dram_tensor("xT", [D, s], f32r, kind="ExternalInput")
    wqT_d = nc.dram_tensor("wqT", [D, 128], f32r, kind="ExternalInput")
    wkT_d = nc.dram_tensor("wkT", [D, 128], f32r, kind="ExternalInput")
    wvT_d = nc.dram_tensor("wvT", [D, 128], f32r, kind="ExternalInput")
    wpT_d = nc.dram_tensor("wpT", [128, D], f32r, kind="ExternalInput")
    ones_d = nc.dram_tensor("ones_in", [128, 64], f32r, kind="ExternalInput")
    out_d = nc.dram_tensor("out_p", [s, D], f32, kind="ExternalOutput")
    if debug_dumps:
        dbg_qT = nc.dram_tensor("dbg_qT", [128, s], f32, kind="ExternalOutput")
        dbg_kT = nc.dram_tensor("dbg_kT", [128, s], f32, kind="ExternalOutput")
        dbg_v65 = nc.dram_tensor("dbg_v65", [128, 2 * 65 * (s // 128)], f32,
                                 kind="ExternalOutput")
        dbg_at = nc.dram_tensor("dbg_at", [128, s], f32, kind="ExternalOutput")

    with ExitStack() as ctx:
        tc = ctx.enter_context(tile.TileContext(nc))
        consts = ctx.enter_context(tc.tile_pool(name="consts", bufs=1))
        big = ctx.enter_context(tc.tile_pool(name="big", bufs=1))
        expool = ctx.enter_context(tc.tile_pool(name="expool", bufs=2))
        recpool = ctx.enter_context(tc.tile_pool(name="recpool", bufs=2))
        outpool = ctx.enter_context(tc.tile_pool(name="outpool", bufs=3))
        mmps = ctx.enter_context(tc.tile_pool(name="mmps", bufs=2, space="PSUM"))
        scps = ctx.enter_context(tc.tile_pool(name="scps", bufs=1, space="PSUM"))
        accps = ctx.enter_context(tc.tile_pool(name="accps", bufs=1, space="PSUM"))

        # ---- constants / persistent SBUF ----
        xT = [big.tile([128, s], f32r, name=f"xT{c}", tag=f"xT{c}") for c in range(ndc)]
        qT = big.tile([128, s], f32r, name="qT", tag="qT")
        kT = big.tile([128, s], f32r, name="kT", tag="kT")
        v65 = [big.tile([128, 65 * nkb_all], f32r, name=f"v65_{h}", tag=f"v65_{h}")
               for h in range(2)]
        attnT = big.tile([128, s], f32r, name="attnT", tag="attnT")
        wq = consts.tile([128, D], f32r, name="wq", tag="wq")
        wk = consts.tile([128, D], f32r, name="wk", tag="wk")
        wv = consts.tile([128, D], f32r, name="wv", tag="wv")
        wp = consts.tile([128, D], f32r, name="wp", tag="wp")
        masks4 = [consts.tile([128, 512], f32, name=f"mask{r}", tag=f"mask{r}")
                  for r in range(kb_per_chunk)]
        ident = consts.tile([128, 128], f32, name="ident", tag="ident")
        onesb = consts.tile([128, 64], f32r, name="onesb", tag="onesb")

        for _rep in range(repeats):
            _emit_body(nc, tc, locals())

        if debug_dumps:
            nc.sync.dma_start(out=dbg_qT.ap(), in_=qT.bitcast(f32))
            nc.sync.dma_start(out=dbg_kT.ap(), in_=kT.bitcast(f32))
            for h in range(2):
                nc.sync.dma_start(
                    out=dbg_v65[:, h * 65 * nkb_all:(h + 1) * 65 * nkb_all],
                    in_=v65[h].bitcast(f32))
            nc.sync.dma_start(out=dbg_at.ap(), in_=attnT.bitcast(f32))

    nc.compile()
    return nc


def _emit_body(nc, tc, env):
    """One full pass of the kernel body (DMAs + all chunks)."""
    import concourse.mybir as mybir
    from concourse.masks import make_identity

    f32 = mybir.dt.float32
    f32r = mybir.dt.float32r
    EXP = mybir.ActivationFunctionType.Exp
    GE = mybir.AluOpType.is_ge
    (s, nqc, nkb_all, ndc, kb_per_chunk, normalize) = (
        env["s"], env["nqc"], env["nkb_all"], env["ndc"],
        env["kb_per_chunk"], env["normalize"])
    (xT_d, wqT_d, wkT_d, wvT_d, wpT_d, ones_d, out_d) = (
        env["xT_d"], env["wqT_d"], env["wkT_d"], env["wvT_d"], env["wpT_d"],
        env["ones_d"], env["out_d"])
    (xT, qT, kT, v65, attnT, wq, wk, wv, wp, masks4, ident, onesb) = (
        env["xT"], env["qT"], env["kT"], env["v65"], env["attnT"], env["wq"],
        env["wk"], env["wv"], env["wp"], env["masks4"], env["ident"],
        env["onesb"])
    (consts, big, expool, recpool, outpool, mmps, scps, accps) = (
        env["consts"], env["big"], env["expool"], env["recpool"],
        env["outpool"], env["mmps"], env["scps"], env["accps"])
    QCHUNK = 512

    if True:
        for c in range(ndc):
            nc.sync.dma_start(out=xT[c], in_=xT_d[c * 128:(c + 1) * 128, :])
        for w_sb, w_d in ((wq, wqT_d), (wk, wkT_d), (wv, wvT_d)):
            for c in range(ndc):
                nc.sync.dma_start(out=w_sb[:, c * 128:(c + 1) * 128],
                                  in_=w_d[c * 128:(c + 1) * 128, :])
        nc.sync.dma_start(out=wp, in_=wpT_d.ap())
        for h in range(2):
            ones_ap = v65[h].rearrange("p (k c) -> p k c", c=65)[:, :, 64]
            nc.sync.dma_start(out=ones_ap, in_=ones_d[:, 0:nkb_all])
        nc.sync.dma_start(out=onesb, in_=ones_d.ap())
        make_identity(nc, ident)
        for r in range(kb_per_chunk):
            # mask[p, f] = 1.0 where f >= p + 128*r else 0.0
            nc.gpsimd.memset(masks4[r], 1.0)
            nc.gpsimd.affine_select(
                out=masks4[r], in_=masks4[r], compare_op=GE, fill=0.0,
                base=-128 * r, channel_multiplier=-1, pattern=[[1, 512]])

        for qc in range(nqc):
            qlo = qc * QCHUNK
            qs = slice(qlo, qlo + QCHUNK)

            # ---- phase 1: project q/k/v for this q-range ----
            for w_sb, dst in ((wq, qT), (wk, kT)):
                ps = mmps.tile([128, QCHUNK], f32, name=f"proj_{qc}", tag="mm")
                for c in range(ndc):
                    nc.tensor.matmul(ps,
                                     lhsT=w_sb[:, c * 128:(c + 1) * 128],
                                     rhs=xT[c][:, qs],
                                     start=(c == 0), stop=(c == ndc - 1))
                nc.vector.tensor_copy(dst[:, qs], ps)
            # natural-layout v rows (x-block stationary): one [128,128] block per kb
            for j in range(kb_per_chunk):
                kb = qc * kb_per_chunk + j
                vp = mmps.tile([128, 128], f32, name=f"vp_{kb}", tag="mm")
                for c in range(ndc):
                    nc.tensor.matmul(vp,
                                     lhsT=xT[c][:, kb * 128:(kb + 1) * 128],
                                     rhs=wv[:, c * 128:(c + 1) * 128],
                                     start=(c == 0), stop=(c == ndc - 1))
                for h in range(2):
                    nc.vector.tensor_copy(v65[h][:, kb * 65:kb * 65 + 64],
                                          vp[:, h * 64:(h + 1) * 64])

            # ---- phase 2: attention over k blocks ----
            nkb = (qc + 1) * kb_per_chunk
            acc = [accps.tile([65, QCHUNK], f32, name=f"acc{h}_{qc}", tag=f"acc{h}")
                   for h in range(2)]
            for kbt in range(nkb // 2):
                for h in range(2):
                    hsl = slice(h * 64, (h + 1) * 64)
                    sc = scps.tile([128, 1024], f32, name=f"sc{h}_{qc}_{kbt}",
                                   tag=f"sc{h}")
                    for j in range(2):
                        kb = kbt * 2 + j
                        nc.tensor.matmul(
                            sc[:, j * 512:(j + 1) * 512],
                            lhsT=kT[hsl, kb * 128:(kb + 1) * 128],
                            rhs=qT[hsl, qs],
                            start=True, stop=True)
                    ex = expool.tile([128, 1024], f32r, name=f"ex{h}_{qc}_{kbt}",
                                     tag=f"ex{h}")
                    nc.scalar.activation(ex, sc, EXP, scale=float(SCALE))
                    for j in range(2):
                        kb = kbt * 2 + j
                        if kb * 128 >= qlo:  # diagonal block: zero where k > q
                            r = kb - qc * kb_per_chunk
                            sl = ex[:, j * 512:(j + 1) * 512]
                            nc.vector.tensor_mul(sl, sl, masks4[r])
                    for j in range(2):
                        kb = kbt * 2 + j
                        nc.tensor.matmul(
                            acc[h],
                            lhsT=v65[h][:, kb * 65:(kb + 1) * 65],
                            rhs=ex[:, j * 512:(j + 1) * 512],
                            start=(kb == 0), stop=(kb == nkb - 1))

            # ---- phase 3: normalize + output projection for this q-chunk ----
            for h in range(2):
                nc.vector.tensor_copy(attnT[h * 64:(h + 1) * 64, qs], acc[h][0:64, :])
            if normalize:
                nfold = QCHUNK // 128
                for h in range(2):
                    hsl = slice(h * 64, (h + 1) * 64)
                    sums_sb = recpool.tile([1, QCHUNK], f32,
                                           name=f"sums{h}_{qc}", tag=f"sums{h}")
                    nc.vector.tensor_copy(sums_sb, acc[h][64:65, :])
                    # fold q across partitions: [1,512] -> [4,128] -> T -> [128,4]
                    rf2 = recpool.tile([nfold, 128], f32, name=f"rf2{h}_{qc}",
                                       tag=f"rf2{h}")
                    nc.sync.dma_start(
                        out=rf2, in_=sums_sb.rearrange("o (c p) -> o c p", p=128))
                    rfp = mmps.tile([128, nfold], f32, name=f"rfp{h}_{qc}",
                                    tag="mm")
                    nc.tensor.transpose(rfp, rf2, ident[0:nfold, 0:nfold])
                    rf = recpool.tile([128, nfold], f32, name=f"rf{h}_{qc}",
                                      tag=f"rf{h}")
                    nc.vector.reciprocal(rf, rfp)
                    diag = recpool.tile([128, QCHUNK], f32r,
                                        name=f"diag{h}_{qc}", tag=f"diag{h}")
                    for jj in range(nfold):
                        nc.vector.tensor_scalar_mul(
                            diag[:, jj * 128:(jj + 1) * 128], ident,
                            rf[:, jj:jj + 1])
                    bc = mmps.tile([64, QCHUNK], f32, name=f"bc{h}_{qc}", tag="mm")
                    nc.tensor.matmul(bc, lhsT=onesb, rhs=diag,
                                     start=True, stop=True)
                    nc.vector.tensor_mul(attnT[hsl, qs], attnT[hsl, qs], bc)
            for j in range(kb_per_chunk):
                qb = qc * kb_per_chunk + j
                pp = mmps.tile([128, D], f32, name=f"pp_{qb}", tag="mm")
                nc.tensor.matmul(pp,
                                 lhsT=attnT[:, qb * 128:(qb + 1) * 128],
                                 rhs=wp, start=True, stop=True)
                ot = outpool.tile([128, D], f32, name=f"ot_{qb}", tag="ot")
                nc.vector.tensor_copy(ot, pp)
                nc.sync.dma_start(out=out_d[qb * 128:(qb + 1) * 128, :], in_=ot)




def _in_maps(x, W_q, W_k, W_v, W_proj):
    maps = []
    for c in range(8):
        b, hp = c // 4, c % 4
        cols = slice(hp * 128, (hp + 1) * 128)
        maps.append({
            "xT": np.ascontiguousarray(x[b].T),
            "wqT": np.ascontiguousarray(W_q.T[:, cols]),
            "wkT": np.ascontiguousarray(W_k.T[:, cols]),
            "wvT": np.ascontiguousarray(W_v.T[:, cols]),
            "wpT": np.ascontiguousarray(W_proj[:, cols].T),
            "ones_in": np.ones((128, 64), dtype=np.float32),
        })
    return maps


def kernel(x, W_q, W_k, W_v, W_proj, b_proj, _trace=False):
    from concourse.bass_utils import run_bass_kernel_spmd

    x = np.asarray(x, dtype=np.float32)
    W_q = np.asarray(W_q, dtype=np.float32)
    W_k = np.asarray(W_k, dtype=np.float32)
    W_v = np.asarray(W_v, dtype=np.float32)
    W_proj = np.asarray(W_proj, dtype=np.float32)
    b_proj = np.asarray(b_proj, dtype=np.float32)

    if "nc" not in _CACHE:
        _CACHE["nc"] = _build()
    nc = _CACHE["nc"]

    res = run_bass_kernel_spmd(nc, _in_maps(x, W_q, W_k, W_v, W_proj),
                               core_ids=list(range(8)), trace=_trace)
    out = np.empty((B, S, D), dtype=np.float32)
    for b in range(B):
        acc = res.results[4 * b]["out_p"].astype(np.float32)
        for j in range(1, 4):
            acc = acc + res.results[4 * b + j]["out_p"]
        out[b] = acc + b_proj
    if _trace:
        _CACHE["last_trace"] = res
    return out
